# revision 20
# baseline (speedup 1.0000x reference)
"""DivergentAttention Trainium2 kernel (8 NeuronCores, Bass/Tile), v5.

Problem: GPT-2 style causal self-attention (B=2, S=2048, D=1024, H=16,
hd=64) where heads 0/1/2 re-weight their attention toward a token region
(first/middle/last third of the sequence) with factor 1.6 and renormalize.

Identity: softmax(s)*m / sum(softmax(s)*m) == softmax(s + log m): the region
reweight folds into an additive per-(head, key) bias on the scores. Scores
are small (|s|<~5) so the max-subtraction pass is skipped.

Sharding: core c handles batch c//4 and heads [4*(c%4), 4*(c%4)+4); host
sums the 8 f32 c_proj partials and adds c_proj_b + c_attn_b_v @ c_proj_w
(the v-bias passes through the attention average exactly, so it folds into
a host-side constant).

v5 design notes (cost-model driven; ACT exp stream is the bottleneck):
  - QKV projection in fp8e4m3 DoubleRow (0.5 cyc/col). q/k use 2-term
    hi/lo compensation (whi+wlo)@hhi -- score noise is dominated by the
    later fp8 re-quantization of q/k anyway; v keeps 3 terms
    (whi@hhi + whi@hlo + wlo@hhi) since v enters the output directly.
    Weights host-prescaled by 64 (fp8e4m3 subnormal cutoff); q/k copies
    descale via DVE tensor_scalar(mult, add); the v-path descale cancels
    in the softmax normalization (denominator ones-column = 64).
  - Inputs are split into per-chunk DRAM tensors (w01|w23|wv hi/lo,
    hidden col-chunks) so every DMA is a whole contiguous tensor with
    >=512B descriptor runs (sub-512B runs pay a 2x DMA latency
    penalty), loaded in critical-path order: bqk, w01 hi+lo, hhi cols
    0:512 -> first score pieces (and the ACT exp stream) start ~5us in.
  - Scores in fp8 DoubleRow: q/k stripes each followed by a ZEROED
    stripe so both slot-1 operands are benign.
  - AV is FLIPPED: out[q-tile 128, 65] = attnT_tile.T @ [v | 1]: 65
    moving cols per (q,k) tile pair; the denominator (col 64) lands on
    the same partitions as q, so normalization is a per-partition
    reciprocal + tensor_scalar_mul on DVE.
  - Per-(head,q-tile) accumulators are packed 7-per-PSUM-bank at 65*4B
    stride; banks are DVE-memset and all AV matmuls use start=False.
    Banks drain only once quiescent.
  - ao[q, hd] is DMA-xbar-transposed to aoT[hd, q] per (q-tile,
    head-pair) for c_proj; c_proj PSUM results are DMAed straight to
    DRAM as f32 (no PSUM->SBUF copy), host does the final reduce. Out
    DMAs ride sync (and scalar only once the exp stream is emitted --
    a waiting DMA occupies its queue's sequencer and would stall exp
    decode). gpsimd/SWDGE never touches PSUM.
  - ACT runs ONLY the exp stream; causal 0/1 diag mask is applied
    post-exp on DVE (bf16, all-SBUF, 2x perf mode); GPSIMD only issues
    SWDGE DMAs for non-PSUM traffic.
  - Emission order IS the dependency order (Tile derives deps from
    program order) and engine-queue priority; every v_tile(t) precedes
    the first tail that reads it (build asserts this invariant).
"""

import numpy as np

import concourse.bass as bass
import concourse.tile as tile
from concourse import mybir
from concourse import bass_utils, bass2jax

# ---------------------------------------------------------------- constants
B, S, D, H, HD = 2, 2048, 1024, 16, 64
NCORES = 8
HPC = 4              # heads per core
GROUPS = 4           # head groups
FOCUS = 1.6
HEAD_REGION = {0: 0, 1: 1, 2: 2}
BF = mybir.dt.bfloat16
F32 = mybir.dt.float32
F8 = mybir.dt.float8e4
NT = S // 128         # 16
KO = D // 128         # 8
CP = 4                # DoubleRow chunk-pairs (256 logical rows each)
WSCALE = 64.0         # c_attn_w prescale so fp8e4m3 stays out of subnormals
QK_TERMS3 = False     # 3-term q/k error compensation (v is always 3-term)
DEBUG_DUMPS = False   # add intermediate-tensor DRAM dumps (debugging only)
# w column order inside w01/w23: [q01|k01] and [q23|k23]; nt (bqk order)
# 0=q01 1=q23 2=k01 3=k23 -> (tensor, col0)
WSLOT = {0: (0, 0), 2: (0, 128), 1: (1, 0), 3: (1, 128)}
# drains are bank-granular: reading a PSUM bank while matmuls still
# accumulate into other columns of it corrupts the accumulation (hw
# read-during-accumulate hazard), so a bank drains only once quiescent.
BATCHES = ((0, 7), (7, 14), (14, 16))
DRAIN_T = {6: 0, 13: 1, 15: 2}               # tail t -> batch index
BANK0 = (0, 7, 14)
LAST_IN_BANK = (True, True, True)

# ------------------------------------------------- walrus multi-wait fixup
# This container's walrus accepts only ONE sync-wait per TPB instruction,
# but Tile attaches one wait per dependency proc. Rewrite the BIR JSON just
# before walrus: hoist all-but-one wait of a multi-wait instruction onto
# standalone same-engine NoOps inserted immediately before it (same-engine
# program order is preserved, so semantics are unchanged).
try:
    import orjson as _json
except ImportError:  # pragma: no cover
    import json as _json

_orig_compile_bir_kernel = bass_utils.compile_bir_kernel
_wfix_counter = [0]


def _fix_bir(bir_json):
    d = _json.loads(bir_json)
    changed = False
    for fn in d.get("functions", []):
        for blk in fn.get("blocks", []):
            out = []
            for inst in blk.get("instructions", []):
                si = inst.get("sync_info")
                if si:
                    waits = si.get("on_wait") or []
                    if len(waits) > 1:
                        changed = True
                        for w in waits[:-1]:
                            _wfix_counter[0] += 1
                            nop = {
                                "engine": inst["engine"],
                                "ins": [],
                                "name": f"I-wfix-{_wfix_counter[0]}",
                                "opcode": "NoOp",
                                "outs": [],
                                "sync_info": {"on_update": [], "on_wait": [w]},
                            }
                            if "debug" in inst:
                                nop["debug"] = inst["debug"]
                            out.append(nop)
                        si["on_wait"] = waits[-1:]
                out.append(inst)
            blk["instructions"] = out
    return _json.dumps(d) if changed else bir_json


def _patched_compile_bir_kernel(bir_json, tmpdir, neff_name="file.neff"):
    return _orig_compile_bir_kernel(_fix_bir(bir_json), tmpdir, neff_name=neff_name)


def _install_waitfix():
    bass_utils.compile_bir_kernel = _patched_compile_bir_kernel
    bass2jax.compile_bir_kernel = _patched_compile_bir_kernel


_install_waitfix()

# ---------------------------------------------------------------- program


def build_program():
    """One SPMD Bass program; per-core differences come in via inputs."""
    nc = bass.Bass()

    # hi/lo fp8 split of hiddenT and 64*c_attn_w, DoubleRow-packed:
    # [p, cp, slot, col] holds logical contraction row 256*cp + 128*slot + p.
    # All inputs are flat [128, bytes] so each DMA is one contiguous tensor.
    w01_hi = nc.dram_tensor("w01_hi", [128, CP * 2 * 256], F8, kind="ExternalInput")
    w01_lo = nc.dram_tensor("w01_lo", [128, CP * 2 * 256], F8, kind="ExternalInput")
    w23_hi = nc.dram_tensor("w23_hi", [128, CP * 2 * 256], F8, kind="ExternalInput")
    w23_lo = nc.dram_tensor("w23_lo", [128, CP * 2 * 256], F8, kind="ExternalInput")
    wv_hi = nc.dram_tensor("wv_hi", [128, CP * 2 * 256], F8, kind="ExternalInput")
    wv_lo = nc.dram_tensor("wv_lo", [128, CP * 2 * 256], F8, kind="ExternalInput")
    h_chunk = {}
    for hl in ("hi", "lo"):
        for i in range(4):
            h_chunk[hl, i] = nc.dram_tensor(
                f"h_{hl}{i}", [128, CP, 2, 512], F8, kind="ExternalInput")
    bqk = nc.dram_tensor("bqk", [128, 4], F32, kind="ExternalInput")
    projw = nc.dram_tensor("projw", [128, 2, D], BF, kind="ExternalInput")
    diag_mask = nc.dram_tensor("diag_mask", [128, 128], BF, kind="ExternalInput")
    logmult = nc.dram_tensor("logmult", [128, HPC, NT], F32, kind="ExternalInput")
    out = nc.dram_tensor("out", [S, D], BF, kind="ExternalOutput")
    if DEBUG_DUMPS:
        dbg_qk = nc.dram_tensor("dbg_qk", [128, 8, S], F8, kind="ExternalOutput")
        dbg_v = nc.dram_tensor("dbg_v", [128, NT, HPC, 65], BF,
                               kind="ExternalOutput")
        dbg_ao = nc.dram_tensor("dbg_ao", [128, NT, 2, 128], BF,
                                kind="ExternalOutput")
        dbg_aoT = nc.dram_tensor("dbg_aoT", [128, NT, 2, 128], BF,
                                 kind="ExternalOutput")
        dbg_at = nc.dram_tensor("dbg_at", [128, 4, 512], BF,
                                kind="ExternalOutput")
        dbg_av = nc.dram_tensor("dbg_av", [128, 455], F32,
                                kind="ExternalOutput")

    with tile.TileContext(nc) as tc:
        with tc.tile_pool(name="persist", bufs=1) as persist, \
             tc.tile_pool(name="p2at", bufs=26) as p2at, \
             tc.tile_pool(name="p2rec", bufs=8) as p2rec, \
             tc.tile_pool(name="p2sc", bufs=2, space="PSUM") as p2sc, \
             tc.tile_pool(name="p2av", bufs=2, space="PSUM") as p2av, \
             tc.tile_pool(name="mix", bufs=2, space="PSUM") as mix:

            # fp8 q/k for DoubleRow scores: each stripe is followed by a
            # ZEROED stripe so both DoubleRow slot-1 operands are benign:
            # the k-side slot-1 weights are 0.0 and the q-side slot-1 data
            # is 0.0 (never a NaN bit pattern from uninitialized SBUF).
            # snt: q01=0, q23=2, k01=4, k23=6; zeros at 1, 3, 5, 7.
            qk_sb = persist.tile([128, 8, S], F8)            # 2 MB
            v_sb = persist.tile([128, NT, HPC, 65], BF)      # ~1.06 MB
            ao_sb = persist.tile([128, NT, 2, 128], BF)      # 1 MB
            aoT_sb = persist.tile([128, NT, 2, 128], BF)     # 1 MB
            bqk_sb = persist.tile([128, 4], F32)
            pw_sb = persist.tile([128, 2, D], BF)
            dm_sb = persist.tile([128, 128], BF)
            lm_sb = persist.tile([128, HPC, NT], F32)
            hhi = persist.tile([128, CP, 2, S], F8)          # 2 MB
            hlo = persist.tile([128, CP, 2, S], F8)          # 2 MB
            # w SBUF tiles: [128, pair(01/23/v), cp, slot, 256]
            w_hi = persist.tile([128, 3, CP, 2, 256], F8)
            w_lo = persist.tile([128, 3, CP, 2, 256], F8)
            o_big = persist.tile([128, NT, D], BF)           # 4 MB out stage
            warm_sb = persist.tile([128, 2, 1024], F8)       # PE warmup zeros

            # v_aug ones column = WSCALE so the w-prescale cancels in the
            # softmax normalization (num and den both carry WSCALE).
            nc.vector.memset(warm_sb[:, :, :].bitcast(F32), 0.0)
            nc.vector.memset(v_sb[:, :, :, 64:65], WSCALE)
            for z in (1, 3, 5, 7):
                nc.vector.memset(qk_sb[:, z, :].bitcast(F32), 0.0)

            def wflat(dst, pair):
                return dst[:, pair, :, :, :].rearrange("p a s c -> p (a s c)")

            # ---- loads (critical-path ordered) ----
            # HWDGE serializes issue (~630ns each) and the DMA engines
            # serialize transfers, so loads are few, whole-tensor DMAs.
            # ALL on sync: a waiting/issuing DMA occupies its queue's
            # sequencer, so the scalar (ACT) queue must stay clear for exp
            # decode and the gpsimd (Pool) queue for the diag masks.
            htile = {"hi": hhi, "lo": hlo}

            def hload(hl, i):
                nc.sync.dma_start(
                    htile[hl][:, :, :, 512 * i:512 * i + 512],
                    h_chunk[hl, i][:, :, :, :])

            nc.sync.dma_start(bqk_sb, bqk[:, :])
            nc.sync.dma_start(wflat(w_hi, 0), w01_hi[:, :])
            nc.sync.dma_start(wflat(w_lo, 0), w01_lo[:, :])
            hload("hi", 0)
            nc.gpsimd.dma_start(dm_sb, diag_mask[:, :])
            nc.gpsimd.dma_start(lm_sb, logmult[:, :, :])
            hload("hi", 1)
            nc.sync.dma_start(wflat(w_hi, 2), wv_hi[:, :])
            nc.sync.dma_start(wflat(w_lo, 2), wv_lo[:, :])
            hload("lo", 0)
            hload("hi", 2)
            hload("hi", 3)
            hload("lo", 1)
            nc.sync.dma_start(wflat(w_hi, 1), w23_hi[:, :])
            nc.sync.dma_start(wflat(w_lo, 1), w23_lo[:, :])
            hload("lo", 2)
            hload("lo", 3)
            nc.sync.dma_start(pw_sb, projw[:, :, :])

            # ---- phase-1 building blocks ----
            _gq = [0]

            def qk_group(nt, sc, eng):
                # one [128, 512] output group; the PSUM->SBUF copy descales
                # by 1/WSCALE and adds the per-partition bias.
                _gq[0] += 1
                pair, c0 = WSLOT[nt]
                ps = mix.tile([128, 512], F32, tag="m", name=f"qk{_gq[0]}")
                terms = ((w_hi, hhi), (w_lo, hhi))
                if QK_TERMS3:
                    terms = terms + ((w_hi, hlo),)
                n = len(terms) * CP
                i = 0
                for wt, ht in terms:
                    for cp in range(CP):
                        nc.tensor.matmul(
                            ps,
                            wt[:, pair, cp, :, c0:c0 + 128],
                            ht[:, cp, :, 512 * sc:512 * sc + 512],
                            start=(i == 0), stop=(i == n - 1),
                            perf_mode=mybir.MatmulPerfMode.DoubleRow,
                        )
                        i += 1
                snt = (0, 2, 4, 6)[nt]   # storage stripe (zeros odd)
                eng.tensor_scalar(
                    qk_sb[:, snt, 512 * sc:512 * sc + 512],
                    ps,
                    1.0 / WSCALE,
                    bqk_sb[:, nt:nt + 1],
                    mybir.AluOpType.mult,
                    mybir.AluOpType.add,
                )

            v_emitted = set()

            def v_tile(st):
                v_emitted.add(st)
                ps = mix.tile([128, 512], F32, tag="m", name=f"v{st}")
                terms = ((w_hi, hhi), (w_lo, hhi), (w_hi, hlo))
                n = len(terms) * CP
                i = 0
                for wt, ht in terms:
                    for cp in range(CP):
                        nc.tensor.matmul(
                            ps[:, 0:256],
                            ht[:, cp, :, 128 * st:128 * st + 128],
                            wt[:, 2, cp, :, :],
                            start=(i == 0), stop=(i == n - 1),
                            perf_mode=mybir.MatmulPerfMode.DoubleRow,
                        )
                        i += 1
                # v bias is folded into the host-side output constant
                # (attention rows sum to 1); GPSIMD cannot read PSUM -> DVE
                nc.vector.tensor_copy(
                    v_sb[:, st, :, 0:64],
                    ps[:, 0:256].rearrange("p (h d) -> p h d", d=64),
                )

            # ---- phase-2 building blocks ----
            av_tiles = {}

            def get_av(lh, tau):
                # hw PSUM accumulation groups don't interleave within a
                # bank: pre-zero the bank and accumulate with start=False.
                bank = 0 if tau < 7 else (1 if tau < 14 else 2)
                if (lh, bank) not in av_tiles:
                    pool = p2av if bank < 2 else mix
                    tag = "av" if bank < 2 else "m"
                    tile_ = pool.tile(
                        [128, 512], F32, tag=tag, name=f"av{lh}{bank}")
                    nc.vector.memset(tile_[:, :], 0.0)
                    av_tiles[(lh, bank)] = tile_
                return av_tiles[(lh, bank)], 7 * (bank > 0) + 7 * (bank > 1)

            def cproj(tau):
                for ec in range(2):
                    # PSUM slots freed upstream become c_proj parallelism:
                    # tau<7 runs while h3 still owns avB + avC (mix slot 1 +
                    # the just-freed p2av slot A); tau>=7 additionally uses
                    # the idle score pool (exp stream is ending).
                    wide = False
                    if tau < 7:
                        pool, tg = (mix, "m") if ec == 0 else (p2av, "av")
                    elif (2 * tau + ec) % 4 == 0:
                        pool, tg = mix, "m"
                    elif (2 * tau + ec) % 4 == 2:
                        pool, tg, wide = p2sc, "sc", True
                    else:
                        pool, tg = p2av, "av"
                    if wide:
                        ps = pool.tile([128, 1024], F32, tag=tg,
                                       name=f"pr{tau}{ec}")[:, 0:512]
                    else:
                        ps = pool.tile([128, 512], F32, tag=tg,
                                       name=f"pr{tau}{ec}")
                    for j in range(2):
                        nc.tensor.matmul(
                            ps,
                            aoT_sb[:, tau, j, :],
                            pw_sb[:, j, 512 * ec:512 * ec + 512],
                            start=(j == 0), stop=(j == 1),
                        )
                    # DMA cannot read PSUM: bounce through the o_big stage.
                    # tau>=7 drains fire after every exp is emitted, so ACT
                    # (idle by then) absorbs half the copies; tau<7 drains
                    # still race the last lh3 exps -> DVE only.
                    dst = o_big[:, tau, 512 * ec:512 * ec + 512]
                    if tau >= 7 and ec == 1:
                        nc.scalar.copy(dst, ps)
                    else:
                        nc.vector.tensor_copy(dst, ps)

            def oflush(t0, t1, oq):
                # one batched out DMA per drain batch: HWDGE issue is
                # globally serialized (~630ns each), so 3 big DMAs beat 32
                # small ones even though the transfer itself serializes.
                oq.dma_start(
                    out[128 * t0:128 * t1, :].rearrange(
                        "(n p) d -> p n d", p=128),
                    o_big[:, t0:t1, :],
                )

            def drain(lh, b):
                # all q-tiles of this batch fully accumulated: reciprocal
                # of the denominator column, per-partition scale into ao_sb.
                t0, t1 = BATCHES[b]
                nb = t1 - t0
                bank = 0 if t0 < 7 else (1 if t0 < 14 else 2)
                if LAST_IN_BANK[b]:
                    av = av_tiles.pop((lh, bank))
                else:
                    av = av_tiles[(lh, bank)]
                av = av[:, 65 * (t0 - BANK0[b]):]
                j, hp = lh // 2, lh % 2
                if DEBUG_DUMPS and lh == 0 and b == 0:
                    avcp = persist.tile([128, 455], F32)
                    nc.vector.tensor_copy(avcp, av[:, 0:455])
                    nc.sync.dma_start(dbg_av[:, :], avcp[:, :])
                rec = p2rec.tile([128, 8], F32, tag="rec")
                den = av[:, 0:65 * nb].rearrange(
                    "p (n c) -> p n c", c=65)[:, :, 64:65]
                nc.vector.reciprocal(rec[:, 0:nb], den)
                for k in range(nb):
                    tau = t0 + k
                    # lh3 drains run at the end of the exp stream, where
                    # ACT has slack; every other normalize mul is on DVE.
                    if lh == 3 and k % 2 == 1:
                        nc.scalar.mul(
                            ao_sb[:, tau, j, 64 * hp:64 * hp + 64],
                            av[:, 65 * k:65 * k + 64],
                            rec[:, k:k + 1],
                        )
                    else:
                        nc.vector.tensor_scalar_mul(
                            ao_sb[:, tau, j, 64 * hp:64 * hp + 64],
                            av[:, 65 * k:65 * k + 64],
                            rec[:, k:k + 1],
                        )
                if hp == 1:
                    # both heads of pair j drained: transpose ao[q, hd] ->
                    # aoT[hd, q] on the DMA xbar; after the last pair,
                    # this q-tile's c_proj is fully unblocked.
                    for k in range(nb):
                        tau = t0 + k
                        nc.sync.dma_start_transpose(
                            aoT_sb[:, tau, j, :], ao_sb[:, tau, j, :])
                    if lh == 3:
                        for k in range(nb):
                            cproj(t0 + k)
                        oflush(t0, t1, nc.scalar if b else nc.sync)

            def tail(lh, t, q0, width, at_sb):
                # exp consumers: causal 0/1 mask on the diagonal block
                # (GPSIMD, all-SBUF) + flipped AV accumulation.
                if q0 == 128 * t:
                    nc.gpsimd.tensor_mul(
                        out=at_sb[:, 0:128], in0=at_sb[:, 0:128],
                        in1=dm_sb,
                    )
                if DEBUG_DUMPS and lh == 0 and t < 4 and q0 < 512:
                    nc.sync.dma_start(dbg_at[:, t, :], at_sb[:, 0:512])
                assert t in v_emitted, (
                    f"tail({lh},{t}) before v_tile({t}): program-order "
                    "dependency violation (reads uninitialized v_sb)")
                v_aug = v_sb[:, t, lh, :]
                for tau in range(q0 // 128, (q0 + width) // 128):
                    av, base = get_av(lh, tau)
                    col = 65 * (tau - base)
                    off = 128 * tau - q0
                    nc.tensor.matmul(
                        av[:, col:col + 65],
                        at_sb[:, off:off + 128],
                        v_aug,
                        start=False, stop=(t == tau),
                        skip_group_check=True,
                    )
                full = q0 + width == (1024 if t < 8 else 2048)
                if full and t in DRAIN_T and (t > 6 or q0 < 1024):
                    drain(lh, DRAIN_T[t])

            pending = []

            def piece(lh, t, hf, q0=None, q1=None):
                if q0 is None:
                    q0 = max(128 * t, 1024 * hf)
                if q1 is None:
                    q1 = 1024 * (hf + 1)
                if q0 >= q1:
                    return
                width = q1 - q0
                bp = 64 * (lh % 2)
                q_nt = 2 * (lh // 2)          # slots (q stripe, zeros)
                k_nt = 4 + 2 * (lh // 2)      # slots (k stripe, zeros)
                lhsT_k = qk_sb[bp:bp + 64, k_nt:k_nt + 2,
                               128 * t:128 * t + 128]
                sc_ps = p2sc.tile([128, 1024], F32, tag="sc")
                off = 0
                while off < width:
                    w512 = min(512, width - off)
                    nc.tensor.matmul(
                        sc_ps[:, off:off + w512],
                        lhsT_k,
                        qk_sb[bp:bp + 64, q_nt:q_nt + 2,
                              q0 + off:q0 + off + w512],
                        start=True, stop=True,
                        perf_mode=mybir.MatmulPerfMode.DoubleRow,
                    )
                    off += w512
                at_sb = p2at.tile([128, 1024], BF, tag="attnT")
                nc.scalar.activation(
                    at_sb[:, :width], sc_ps[:, :width],
                    mybir.ActivationFunctionType.Exp,
                    bias=lm_sb[:, lh, t:t + 1], scale=0.125,
                )
                pending.append((lh, t, q0, width, at_sb))
                if len(pending) > 9:
                    tail(*pending.pop(0))

            # ---- interleaved emission: program order is engine priority ----
            # PE p-state warmup: the cost model runs PE at 0.65/1.2 GHz
            # until it has been busy ~3us, and the ramp clock resets on
            # idle. Zero-input dummy matmuls keep PE busy from ~1.3us so
            # the first real matmuls (~7us, DMA-bound) run at 2.4 GHz.
            warm_ps = p2sc.tile([128, 1024], F32, tag="sc", name="warm")
            for _ in range(12):
                nc.tensor.matmul(
                    warm_ps[:, 0:512],
                    warm_sb[0:64, :, 0:128],
                    warm_sb[0:64, :, 0:512],
                    start=True, stop=True,
                    perf_mode=mybir.MatmulPerfMode.DoubleRow,
                )
            V = nc.vector
            # NOTE: tails (av matmuls) consume v_sb, and Tile derives
            # dependencies from program order -- every v_tile(st) must be
            # emitted BEFORE the first tail that reads v_sb[:, st].
            # With pending depth 10, tail of piece i pops at piece i+10.
            # v tiles sit a bit later than in v3 so the hlo DMAs (behind
            # hhi in the load order) have landed by the time the in-order
            # PE stream reaches them.
            qk_group(2, 0, V)                 # k01 cols 0:512
            qk_group(0, 0, V)                 # q01 cols 0:512
            for t in range(4):
                piece(0, t, 0, q1=512)        # needs only the two groups above
            qk_group(0, 1, V)                 # q01 cols 512:1024
            for t in range(4):
                piece(0, t, 0, q0=512)
            qk_group(2, 1, V)
            piece(0, 4, 0)
            v_tile(0)
            v_tile(1)
            piece(0, 5, 0)
            piece(0, 6, 0)
            v_tile(2)
            v_tile(3)
            piece(0, 7, 0)
            qk_group(0, 2, V)
            qk_group(0, 3, V)
            for t in range(2):
                piece(0, t, 1)
            v_tile(4)
            v_tile(5)
            for t in range(2, 4):
                piece(0, t, 1)
            v_tile(6)
            v_tile(7)
            for t in range(4, 6):
                piece(0, t, 1)
            qk_group(2, 2, V)
            qk_group(2, 3, V)
            for t in range(6, 10):
                piece(0, t, 1)
            for st in range(8, 11):
                v_tile(st)
            for t in range(10, 16):
                piece(0, t, 1)
            for st in range(11, 16):
                v_tile(st)
            for t in range(8):
                piece(1, t, 0)
            for t in range(16):
                piece(1, t, 1)
            qk_group(1, 0, V)                 # q23 cols 0:512
            qk_group(3, 0, V)                 # k23 cols 0:512
            qk_group(1, 1, V)
            qk_group(3, 1, V)
            for t in range(8):
                piece(2, t, 0)
            qk_group(1, 2, V)
            qk_group(3, 2, V)
            qk_group(1, 3, V)
            qk_group(3, 3, V)
            for t in range(16):
                piece(2, t, 1)
            for t in range(8):
                piece(3, t, 0)
            for t in range(16):
                piece(3, t, 1)
            for pc in pending:
                tail(*pc)
            pending.clear()
            if DEBUG_DUMPS:
                nc.sync.dma_start(dbg_qk[:, :, :], qk_sb[:, :, :])
                nc.sync.dma_start(dbg_v[:, :, :, :], v_sb[:, :, :, :])
                nc.sync.dma_start(dbg_ao[:, :, :, :], ao_sb[:, :, :, :])
                nc.sync.dma_start(dbg_aoT[:, :, :, :], aoT_sb[:, :, :, :])
    return nc


_NC = None


def _get_nc():
    global _NC
    if _NC is None:
        _NC = build_program()
    return _NC


# ---------------------------------------------------------------- host prep

def make_in_maps(hidden_states, c_attn_w, c_attn_b, c_proj_w):
    import ml_dtypes
    bf16 = ml_dtypes.bfloat16
    f8 = mybir.dt.np(F8)

    def pack_hilo(arr):
        # [1024, N] f32 -> hi/lo fp8 DoubleRow packs [128, CP, 2, N]
        hi = arr.astype(f8)
        lo = (arr - hi.astype(np.float32)).astype(f8)
        out = []
        for part in (hi, lo):
            p = part.reshape(CP, 2, 128, -1).transpose(2, 0, 1, 3)
            out.append(np.ascontiguousarray(p))
        return out

    first_end = S // 3
    second_end = 2 * S // 3
    pos = np.arange(S)
    regions = [pos < first_end,
               (pos >= first_end) & (pos < second_end),
               pos >= second_end]
    mult = np.ones((H, S), dtype=np.float64)
    for h, r in HEAD_REGION.items():
        mult[h] = 1.0 + (FOCUS - 1.0) * regions[r].astype(np.float64)
    logm = np.log(mult).astype(np.float32)  # [H, S]

    p = np.arange(128)[:, None]
    j = np.arange(128)[None, :]
    diag = (j >= p).astype(np.float32)  # 0/1 keep-mask, applied post-exp

    in_maps = []
    for c in range(NCORES):
        b, g = divmod(c, GROUPS)
        h0 = HPC * g
        cs = slice(256 * g, 256 * g + 256)
        wq = c_attn_w[:, cs]
        wk = c_attn_w[:, 1024:2048][:, cs]
        wv = c_attn_w[:, 2048:3072][:, cs]
        # w column blocks: w01=[q01|k01], w23=[q23|k23], wv
        w01 = np.concatenate([wq[:, 0:128], wk[:, 0:128]], axis=1)
        w23 = np.concatenate([wq[:, 128:256], wk[:, 128:256]], axis=1)
        bqk = np.concatenate(
            [c_attn_b[cs], c_attn_b[1024:2048][cs]]
        ).reshape(4, 128).T.copy().astype(np.float32)
        # pw2[p, j, e]: head pair j=(2j, 2j+1); p<64 -> head 2j row p,
        # p>=64 -> head 2j+1 row p-64  (matches aoT partition layout)
        pw = c_proj_w[64 * h0:64 * h0 + 256, :].reshape(2, 128, D)
        pw = np.ascontiguousarray(pw.transpose(1, 0, 2))
        lm = logm[h0:h0 + HPC].reshape(HPC, S // 128, 128)
        lm = np.ascontiguousarray(lm.transpose(2, 0, 1)).astype(np.float32)
        h_hi, h_lo = pack_hilo(np.ascontiguousarray(hidden_states[b].T))
        im = {"bqk": bqk, "projw": pw.astype(bf16),
              "diag_mask": diag.astype(bf16), "logmult": lm}
        for name, wblk in (("w01", w01), ("w23", w23), ("wv", wv)):
            whi_, wlo_ = pack_hilo(WSCALE * wblk)
            im[f"{name}_hi"] = np.ascontiguousarray(whi_.reshape(128, -1))
            im[f"{name}_lo"] = np.ascontiguousarray(wlo_.reshape(128, -1))
        for i in range(4):
            cols = slice(512 * i, 512 * i + 512)
            im[f"h_hi{i}"] = np.ascontiguousarray(h_hi[:, :, :, cols])
            im[f"h_lo{i}"] = np.ascontiguousarray(h_lo[:, :, :, cols])
        in_maps.append(im)
    return in_maps


def run_cores(in_maps, trace=False, **kw):
    from concourse.bass_utils import run_bass_kernel_spmd
    nc = _get_nc()
    return run_bass_kernel_spmd(nc, in_maps, core_ids=list(range(NCORES)),
                                trace=trace, **kw)


def kernel(hidden_states, c_attn_w, c_attn_b, c_proj_w, c_proj_b):
    hidden_states = np.asarray(hidden_states, dtype=np.float32)
    c_attn_w = np.asarray(c_attn_w, dtype=np.float32)
    c_attn_b = np.asarray(c_attn_b, dtype=np.float32)
    c_proj_w = np.asarray(c_proj_w, dtype=np.float32)
    c_proj_b = np.asarray(c_proj_b, dtype=np.float32)

    in_maps = make_in_maps(hidden_states, c_attn_w, c_attn_b, c_proj_w)
    res = run_cores(in_maps)
    out = np.zeros((B, S, D), dtype=np.float32)
    for c in range(NCORES):
        out[c // GROUPS] += np.asarray(res.results[c]["out"], dtype=np.float32)
    # v-bias passes through the attention average exactly (rows sum to 1),
    # so it folds into a constant along with c_proj_b.
    out += (c_proj_b + c_attn_b[2048:3072] @ c_proj_w)[None, None, :]
    return out


# revision 23
# speedup vs baseline: 1.0269x; 1.0269x over previous
"""DivergentAttention Trainium2 kernel (8 NeuronCores, Bass/Tile), v5.

Problem: GPT-2 style causal self-attention (B=2, S=2048, D=1024, H=16,
hd=64) where heads 0/1/2 re-weight their attention toward a token region
(first/middle/last third of the sequence) with factor 1.6 and renormalize.

Identity: softmax(s)*m / sum(softmax(s)*m) == softmax(s + log m): the region
reweight folds into an additive per-(head, key) bias on the scores. Scores
are small (|s|<~5) so the max-subtraction pass is skipped.

Sharding: core c handles batch c//4 and heads [4*(c%4), 4*(c%4)+4); host
sums the 8 f32 c_proj partials and adds c_proj_b + c_attn_b_v @ c_proj_w
(the v-bias passes through the attention average exactly, so it folds into
a host-side constant).

v5 design notes (cost-model driven; ACT exp stream is the bottleneck):
  - QKV projection in fp8e4m3 DoubleRow (0.5 cyc/col). q/k use 2-term
    hi/lo compensation (whi+wlo)@hhi -- score noise is dominated by the
    later fp8 re-quantization of q/k anyway; v keeps 3 terms
    (whi@hhi + whi@hlo + wlo@hhi) since v enters the output directly.
    Weights host-prescaled by 64 (fp8e4m3 subnormal cutoff); q/k copies
    descale via DVE tensor_scalar(mult, add); the v-path descale cancels
    in the softmax normalization (denominator ones-column = 64).
  - Inputs are split into per-chunk DRAM tensors (w01|w23|wv hi/lo,
    hidden col-chunks) so every DMA is a whole contiguous tensor with
    >=512B descriptor runs (sub-512B runs pay a 2x DMA latency
    penalty), loaded in critical-path order: bqk, w01 hi+lo, hhi cols
    0:512 -> first score pieces (and the ACT exp stream) start ~5us in.
  - Scores in fp8 DoubleRow: q/k stripes each followed by a ZEROED
    stripe so both slot-1 operands are benign.
  - AV is FLIPPED: out[q-tile 128, 65] = attnT_tile.T @ [v | 1]: 65
    moving cols per (q,k) tile pair; the denominator (col 64) lands on
    the same partitions as q, so normalization is a per-partition
    reciprocal + tensor_scalar_mul on DVE.
  - Per-(head,q-tile) accumulators are packed 7-per-PSUM-bank at 65*4B
    stride; banks are DVE-memset and all AV matmuls use start=False.
    Banks drain only once quiescent.
  - ao[q, hd] is DMA-xbar-transposed to aoT[hd, q] per (q-tile,
    head-pair) for c_proj; c_proj PSUM results are DMAed straight to
    DRAM as f32 (no PSUM->SBUF copy), host does the final reduce. Out
    DMAs ride sync (and scalar only once the exp stream is emitted --
    a waiting DMA occupies its queue's sequencer and would stall exp
    decode). gpsimd/SWDGE never touches PSUM.
  - ACT runs ONLY the exp stream; causal 0/1 diag mask is applied
    post-exp on DVE (bf16, all-SBUF, 2x perf mode); GPSIMD only issues
    SWDGE DMAs for non-PSUM traffic.
  - Emission order IS the dependency order (Tile derives deps from
    program order) and engine-queue priority; every v_tile(t) precedes
    the first tail that reads it (build asserts this invariant).
"""

import numpy as np

import concourse.bass as bass
import concourse.tile as tile
from concourse import mybir
from concourse import bass_utils, bass2jax

# ---------------------------------------------------------------- constants
B, S, D, H, HD = 2, 2048, 1024, 16, 64
NCORES = 8
HPC = 4              # heads per core
GROUPS = 4           # head groups
FOCUS = 1.6
HEAD_REGION = {0: 0, 1: 1, 2: 2}
BF = mybir.dt.bfloat16
F32 = mybir.dt.float32
F8 = mybir.dt.float8e4
NT = S // 128         # 16
KO = D // 128         # 8
CP = 4                # DoubleRow chunk-pairs (256 logical rows each)
WSCALE = 64.0         # c_attn_w prescale so fp8e4m3 stays out of subnormals
QK_TERMS3 = False     # 3-term q/k error compensation (v is always 3-term)
DEBUG_DUMPS = False   # add intermediate-tensor DRAM dumps (debugging only)
# w column order inside w01/w23: [q01|k01] and [q23|k23]; nt (bqk order)
# 0=q01 1=q23 2=k01 3=k23 -> (tensor, col0)
WSLOT = {0: (0, 0), 2: (0, 128), 1: (1, 0), 3: (1, 128)}
# drains are bank-granular: reading a PSUM bank while matmuls still
# accumulate into other columns of it corrupts the accumulation (hw
# read-during-accumulate hazard), so a bank drains only once quiescent.
BATCHES = ((0, 7), (7, 14), (14, 16))
DRAIN_T = {6: 0, 13: 1, 15: 2}               # tail t -> batch index
BANK0 = (0, 7, 14)
LAST_IN_BANK = (True, True, True)

# ------------------------------------------------- walrus multi-wait fixup
# This container's walrus accepts only ONE sync-wait per TPB instruction,
# but Tile attaches one wait per dependency proc. Rewrite the BIR JSON just
# before walrus: hoist all-but-one wait of a multi-wait instruction onto
# standalone same-engine NoOps inserted immediately before it (same-engine
# program order is preserved, so semantics are unchanged).
try:
    import orjson as _json
except ImportError:  # pragma: no cover
    import json as _json

_orig_compile_bir_kernel = bass_utils.compile_bir_kernel
_wfix_counter = [0]


def _fix_bir(bir_json):
    d = _json.loads(bir_json)
    changed = False
    for fn in d.get("functions", []):
        for blk in fn.get("blocks", []):
            out = []
            for inst in blk.get("instructions", []):
                si = inst.get("sync_info")
                if si:
                    waits = si.get("on_wait") or []
                    if len(waits) > 1:
                        changed = True
                        for w in waits[:-1]:
                            _wfix_counter[0] += 1
                            nop = {
                                "engine": inst["engine"],
                                "ins": [],
                                "name": f"I-wfix-{_wfix_counter[0]}",
                                "opcode": "NoOp",
                                "outs": [],
                                "sync_info": {"on_update": [], "on_wait": [w]},
                            }
                            if "debug" in inst:
                                nop["debug"] = inst["debug"]
                            out.append(nop)
                        si["on_wait"] = waits[-1:]
                out.append(inst)
            blk["instructions"] = out
    return _json.dumps(d) if changed else bir_json


def _patched_compile_bir_kernel(bir_json, tmpdir, neff_name="file.neff"):
    return _orig_compile_bir_kernel(_fix_bir(bir_json), tmpdir, neff_name=neff_name)


def _install_waitfix():
    bass_utils.compile_bir_kernel = _patched_compile_bir_kernel
    bass2jax.compile_bir_kernel = _patched_compile_bir_kernel


_install_waitfix()

# ---------------------------------------------------------------- program


def build_program():
    """One SPMD Bass program; per-core differences come in via inputs."""
    nc = bass.Bass()

    # hi/lo fp8 split of hiddenT and 64*c_attn_w, DoubleRow-packed:
    # [p, cp, slot, col] holds logical contraction row 256*cp + 128*slot + p.
    # All inputs are flat [128, bytes] so each DMA is one contiguous tensor.
    w01_hi = nc.dram_tensor("w01_hi", [128, CP * 2 * 256], F8, kind="ExternalInput")
    w01_lo = nc.dram_tensor("w01_lo", [128, CP * 2 * 256], F8, kind="ExternalInput")
    w23_hi = nc.dram_tensor("w23_hi", [128, CP * 2 * 256], F8, kind="ExternalInput")
    w23_lo = nc.dram_tensor("w23_lo", [128, CP * 2 * 256], F8, kind="ExternalInput")
    wv_hi = nc.dram_tensor("wv_hi", [128, CP * 2 * 256], F8, kind="ExternalInput")
    wv_lo = nc.dram_tensor("wv_lo", [128, CP * 2 * 256], F8, kind="ExternalInput")
    h_chunk = {}
    for hl in ("hi", "lo"):
        for i in range(4):
            h_chunk[hl, i] = nc.dram_tensor(
                f"h_{hl}{i}", [128, CP, 2, 512], F8, kind="ExternalInput")
    bqk = nc.dram_tensor("bqk", [128, 4], F32, kind="ExternalInput")
    projw = nc.dram_tensor("projw", [128, 2, D], BF, kind="ExternalInput")
    diag_mask = nc.dram_tensor("diag_mask", [128, 128], BF, kind="ExternalInput")
    logmult = nc.dram_tensor("logmult", [128, HPC, NT], F32, kind="ExternalInput")
    out = nc.dram_tensor("out", [S, D], BF, kind="ExternalOutput")
    if DEBUG_DUMPS:
        dbg_qk = nc.dram_tensor("dbg_qk", [128, 8, S], F8, kind="ExternalOutput")
        dbg_v = nc.dram_tensor("dbg_v", [128, NT, HPC, 65], BF,
                               kind="ExternalOutput")
        dbg_ao = nc.dram_tensor("dbg_ao", [128, NT, 2, 128], BF,
                                kind="ExternalOutput")
        dbg_aoT = nc.dram_tensor("dbg_aoT", [128, NT, 2, 128], BF,
                                 kind="ExternalOutput")
        dbg_at = nc.dram_tensor("dbg_at", [128, 4, 512], BF,
                                kind="ExternalOutput")
        dbg_av = nc.dram_tensor("dbg_av", [128, 455], F32,
                                kind="ExternalOutput")

    with tile.TileContext(nc) as tc:
        with tc.tile_pool(name="persist", bufs=1) as persist, \
             tc.tile_pool(name="p2at", bufs=26) as p2at, \
             tc.tile_pool(name="p2rec", bufs=8) as p2rec, \
             tc.tile_pool(name="p2sc", bufs=2, space="PSUM") as p2sc, \
             tc.tile_pool(name="p2av", bufs=2, space="PSUM") as p2av, \
             tc.tile_pool(name="mix", bufs=2, space="PSUM") as mix:

            # fp8 q/k for DoubleRow scores: each stripe is followed by a
            # ZEROED stripe so both DoubleRow slot-1 operands are benign:
            # the k-side slot-1 weights are 0.0 and the q-side slot-1 data
            # is 0.0 (never a NaN bit pattern from uninitialized SBUF).
            # snt: q01=0, q23=2, k01=4, k23=6; zeros at 1, 3, 5, 7.
            qk_sb = persist.tile([128, 8, S], F8)            # 2 MB
            v_sb = persist.tile([128, NT, HPC, 65], BF)      # ~1.06 MB
            ao_sb = persist.tile([128, NT, 2, 128], BF)      # 1 MB
            aoT_sb = persist.tile([128, NT, 2, 128], BF)     # 1 MB
            bqk_sb = persist.tile([128, 4], F32)
            pw_sb = persist.tile([128, 2, D], BF)
            dm_sb = persist.tile([128, 128], BF)
            lm_sb = persist.tile([128, HPC, NT], F32)
            hhi = persist.tile([128, CP, 2, S], F8)          # 2 MB
            hlo = persist.tile([128, CP, 2, S], F8)          # 2 MB
            # w SBUF tiles: [128, pair(01/23/v), cp, slot, 256]
            w_hi = persist.tile([128, 3, CP, 2, 256], F8)
            w_lo = persist.tile([128, 3, CP, 2, 256], F8)
            o_big = persist.tile([128, NT, D], BF)           # 4 MB out stage
            warm_sb = persist.tile([128, 2, 1024], F8)       # PE warmup zeros

            # v_aug ones column = WSCALE so the w-prescale cancels in the
            # softmax normalization (num and den both carry WSCALE).
            nc.vector.memset(warm_sb[:, :, :].bitcast(F32), 0.0)
            nc.vector.memset(v_sb[:, :, :, 64:65], WSCALE)
            for z in (1, 3, 5, 7):
                nc.vector.memset(qk_sb[:, z, :].bitcast(F32), 0.0)

            def wflat(dst, pair):
                return dst[:, pair, :, :, :].rearrange("p a s c -> p (a s c)")

            # ---- loads (critical-path ordered) ----
            # HWDGE serializes issue (~630ns each) and the DMA engines
            # serialize transfers, so loads are few, whole-tensor DMAs.
            # ALL on sync: a waiting/issuing DMA occupies its queue's
            # sequencer, so the scalar (ACT) queue must stay clear for exp
            # decode and the gpsimd (Pool) queue for the diag masks.
            htile = {"hi": hhi, "lo": hlo}

            def hload(hl, i):
                nc.sync.dma_start(
                    htile[hl][:, :, :, 512 * i:512 * i + 512],
                    h_chunk[hl, i][:, :, :, :])

            nc.sync.dma_start(bqk_sb, bqk[:, :])
            nc.sync.dma_start(wflat(w_hi, 0), w01_hi[:, :])
            nc.sync.dma_start(wflat(w_lo, 0), w01_lo[:, :])
            hload("hi", 0)
            nc.gpsimd.dma_start(dm_sb, diag_mask[:, :])
            nc.gpsimd.dma_start(lm_sb, logmult[:, :, :])
            hload("hi", 1)
            nc.sync.dma_start(wflat(w_hi, 2), wv_hi[:, :])
            nc.sync.dma_start(wflat(w_lo, 2), wv_lo[:, :])
            hload("lo", 0)
            hload("hi", 2)
            hload("hi", 3)
            hload("lo", 1)
            nc.sync.dma_start(wflat(w_hi, 1), w23_hi[:, :])
            nc.sync.dma_start(wflat(w_lo, 1), w23_lo[:, :])
            hload("lo", 2)
            hload("lo", 3)
            nc.sync.dma_start(pw_sb, projw[:, :, :])

            # ---- phase-1 building blocks ----
            _gq = [0]

            def qk_group(nt, sc, eng):
                # one [128, 512] output group; the PSUM->SBUF copy descales
                # by 1/WSCALE and adds the per-partition bias.
                _gq[0] += 1
                pair, c0 = WSLOT[nt]
                ps = mix.tile([128, 512], F32, tag="m", name=f"qk{_gq[0]}")
                terms = ((w_hi, hhi), (w_lo, hhi))
                if QK_TERMS3:
                    terms = terms + ((w_hi, hlo),)
                n = len(terms) * CP
                i = 0
                for wt, ht in terms:
                    for cp in range(CP):
                        nc.tensor.matmul(
                            ps,
                            wt[:, pair, cp, :, c0:c0 + 128],
                            ht[:, cp, :, 512 * sc:512 * sc + 512],
                            start=(i == 0), stop=(i == n - 1),
                            perf_mode=mybir.MatmulPerfMode.DoubleRow,
                        )
                        i += 1
                snt = (0, 2, 4, 6)[nt]   # storage stripe (zeros odd)
                eng.tensor_scalar(
                    qk_sb[:, snt, 512 * sc:512 * sc + 512],
                    ps,
                    1.0 / WSCALE,
                    bqk_sb[:, nt:nt + 1],
                    mybir.AluOpType.mult,
                    mybir.AluOpType.add,
                )

            v_emitted = set()

            def v_tile(st):
                v_emitted.add(st)
                ps = mix.tile([128, 512], F32, tag="m", name=f"v{st}")
                terms = ((w_hi, hhi), (w_lo, hhi), (w_hi, hlo))
                n = len(terms) * CP
                i = 0
                for wt, ht in terms:
                    for cp in range(CP):
                        nc.tensor.matmul(
                            ps[:, 0:256],
                            ht[:, cp, :, 128 * st:128 * st + 128],
                            wt[:, 2, cp, :, :],
                            start=(i == 0), stop=(i == n - 1),
                            perf_mode=mybir.MatmulPerfMode.DoubleRow,
                        )
                        i += 1
                # v bias is folded into the host-side output constant
                # (attention rows sum to 1); GPSIMD cannot read PSUM -> DVE
                nc.vector.tensor_copy(
                    v_sb[:, st, :, 0:64],
                    ps[:, 0:256].rearrange("p (h d) -> p h d", d=64),
                )

            # ---- phase-2 building blocks ----
            av_tiles = {}

            def get_av(lh, tau):
                # hw PSUM accumulation groups don't interleave within a
                # bank: pre-zero the bank and accumulate with start=False.
                bank = 0 if tau < 7 else (1 if tau < 14 else 2)
                if (lh, bank) not in av_tiles:
                    pool = p2av if bank < 2 else mix
                    tag = "av" if bank < 2 else "m"
                    tile_ = pool.tile(
                        [128, 512], F32, tag=tag, name=f"av{lh}{bank}")
                    nc.vector.memset(tile_[:, :], 0.0)
                    av_tiles[(lh, bank)] = tile_
                return av_tiles[(lh, bank)], 7 * (bank > 0) + 7 * (bank > 1)

            def cproj(tau):
                for ec in range(2):
                    # PSUM slots freed upstream become c_proj parallelism:
                    # tau<7 runs while h3 still owns avB + avC (mix slot 1 +
                    # the just-freed p2av slot A); tau>=7 additionally uses
                    # the idle score pool (exp stream is ending).
                    wide = False
                    if tau < 7:
                        pool, tg = (mix, "m") if ec == 0 else (p2av, "av")
                    elif (2 * tau + ec) % 4 == 0:
                        pool, tg = mix, "m"
                    elif (2 * tau + ec) % 4 == 2:
                        pool, tg, wide = p2sc, "sc", True
                    else:
                        pool, tg = p2av, "av"
                    if wide:
                        ps = pool.tile([128, 1024], F32, tag=tg,
                                       name=f"pr{tau}{ec}")[:, 0:512]
                    else:
                        ps = pool.tile([128, 512], F32, tag=tg,
                                       name=f"pr{tau}{ec}")
                    for j in range(2):
                        nc.tensor.matmul(
                            ps,
                            aoT_sb[:, tau, j, :],
                            pw_sb[:, j, 512 * ec:512 * ec + 512],
                            start=(j == 0), stop=(j == 1),
                        )
                    # DMA cannot read PSUM: bounce through the o_big stage.
                    # tau>=7 drains fire after every exp is emitted, so ACT
                    # (idle by then) absorbs half the copies; tau<7 drains
                    # still race the last lh3 exps -> DVE only.
                    dst = o_big[:, tau, 512 * ec:512 * ec + 512]
                    if tau >= 7 and ec == 1:
                        nc.scalar.copy(dst, ps)
                    else:
                        nc.vector.tensor_copy(dst, ps)

            def oflush(t0, t1, oq):
                # one batched out DMA per drain batch: HWDGE issue is
                # globally serialized (~630ns each), so 3 big DMAs beat 32
                # small ones even though the transfer itself serializes.
                oq.dma_start(
                    out[128 * t0:128 * t1, :].rearrange(
                        "(n p) d -> p n d", p=128),
                    o_big[:, t0:t1, :],
                )

            def drain(lh, b):
                # all q-tiles of this batch fully accumulated: reciprocal
                # of the denominator column, per-partition scale into ao_sb.
                t0, t1 = BATCHES[b]
                nb = t1 - t0
                bank = 0 if t0 < 7 else (1 if t0 < 14 else 2)
                if LAST_IN_BANK[b]:
                    av = av_tiles.pop((lh, bank))
                else:
                    av = av_tiles[(lh, bank)]
                av = av[:, 65 * (t0 - BANK0[b]):]
                j, hp = lh // 2, lh % 2
                if DEBUG_DUMPS and lh == 0 and b == 0:
                    avcp = persist.tile([128, 455], F32)
                    nc.vector.tensor_copy(avcp, av[:, 0:455])
                    nc.sync.dma_start(dbg_av[:, :], avcp[:, :])
                rec = p2rec.tile([128, 8], F32, tag="rec")
                den = av[:, 0:65 * nb].rearrange(
                    "p (n c) -> p n c", c=65)[:, :, 64:65]
                nc.vector.reciprocal(rec[:, 0:nb], den)
                for k in range(nb):
                    tau = t0 + k
                    # lh3 late drains run after the exp stream has ended,
                    # so ACT can absorb half those normalize multiplies;
                    # bank-0 drains still race the last exps -> DVE.
                    if lh == 3 and b >= 1 and k % 2 == 1:
                        nc.scalar.mul(
                            ao_sb[:, tau, j, 64 * hp:64 * hp + 64],
                            av[:, 65 * k:65 * k + 64],
                            rec[:, k:k + 1],
                        )
                    else:
                        nc.vector.tensor_scalar_mul(
                            ao_sb[:, tau, j, 64 * hp:64 * hp + 64],
                            av[:, 65 * k:65 * k + 64],
                            rec[:, k:k + 1],
                        )
                if hp == 1:
                    # both heads of pair j drained: transpose ao[q, hd] ->
                    # aoT[hd, q] on the DMA xbar; after the last pair,
                    # this q-tile's c_proj is fully unblocked.
                    for k in range(nb):
                        tau = t0 + k
                        nc.sync.dma_start_transpose(
                            aoT_sb[:, tau, j, :], ao_sb[:, tau, j, :])
                    if lh == 3:
                        for k in range(nb):
                            cproj(t0 + k)

            def tail(lh, t, q0, width, at_sb):
                # exp consumers: causal 0/1 mask on the diagonal block
                # (GPSIMD, all-SBUF) + flipped AV accumulation.
                if q0 == 128 * t:
                    nc.gpsimd.tensor_mul(
                        out=at_sb[:, 0:128], in0=at_sb[:, 0:128],
                        in1=dm_sb,
                    )
                if DEBUG_DUMPS and lh == 0 and t < 4 and q0 < 512:
                    nc.sync.dma_start(dbg_at[:, t, :], at_sb[:, 0:512])
                assert t in v_emitted, (
                    f"tail({lh},{t}) before v_tile({t}): program-order "
                    "dependency violation (reads uninitialized v_sb)")
                v_aug = v_sb[:, t, lh, :]
                for tau in range(q0 // 128, (q0 + width) // 128):
                    av, base = get_av(lh, tau)
                    col = 65 * (tau - base)
                    off = 128 * tau - q0
                    nc.tensor.matmul(
                        av[:, col:col + 65],
                        at_sb[:, off:off + 128],
                        v_aug,
                        start=False, stop=(t == tau),
                        skip_group_check=True,
                    )
                full = q0 + width == (1024 if t < 8 else 2048)
                if full and t in DRAIN_T and (t > 6 or q0 < 1024):
                    drain(lh, DRAIN_T[t])

            pending = []

            def piece(lh, t, hf, q0=None, q1=None):
                if q0 is None:
                    q0 = max(128 * t, 1024 * hf)
                if q1 is None:
                    q1 = 1024 * (hf + 1)
                if q0 >= q1:
                    return
                width = q1 - q0
                bp = 64 * (lh % 2)
                q_nt = 2 * (lh // 2)          # slots (q stripe, zeros)
                k_nt = 4 + 2 * (lh // 2)      # slots (k stripe, zeros)
                lhsT_k = qk_sb[bp:bp + 64, k_nt:k_nt + 2,
                               128 * t:128 * t + 128]
                sc_ps = p2sc.tile([128, 1024], F32, tag="sc")
                off = 0
                while off < width:
                    w512 = min(512, width - off)
                    nc.tensor.matmul(
                        sc_ps[:, off:off + w512],
                        lhsT_k,
                        qk_sb[bp:bp + 64, q_nt:q_nt + 2,
                              q0 + off:q0 + off + w512],
                        start=True, stop=True,
                        perf_mode=mybir.MatmulPerfMode.DoubleRow,
                    )
                    off += w512
                at_sb = p2at.tile([128, 1024], BF, tag="attnT")
                nc.scalar.activation(
                    at_sb[:, :width], sc_ps[:, :width],
                    mybir.ActivationFunctionType.Exp,
                    bias=lm_sb[:, lh, t:t + 1], scale=0.125,
                )
                pending.append((lh, t, q0, width, at_sb))
                if len(pending) > 9:
                    tail(*pending.pop(0))

            # ---- interleaved emission: program order is engine priority ----
            # PE p-state warmup: the cost model runs PE at 0.65/1.2 GHz
            # until it has been busy ~3us, and the ramp clock resets on
            # idle. Zero-input dummy matmuls keep PE busy from ~1.3us so
            # the first real matmuls (~7us, DMA-bound) run at 2.4 GHz.
            warm_ps = p2sc.tile([128, 1024], F32, tag="sc", name="warm")
            for _ in range(12):
                nc.tensor.matmul(
                    warm_ps[:, 0:512],
                    warm_sb[0:64, :, 0:128],
                    warm_sb[0:64, :, 0:512],
                    start=True, stop=True,
                    perf_mode=mybir.MatmulPerfMode.DoubleRow,
                )
            V = nc.vector
            # NOTE: tails (av matmuls) consume v_sb, and Tile derives
            # dependencies from program order -- every v_tile(st) must be
            # emitted BEFORE the first tail that reads v_sb[:, st].
            # With pending depth 10, tail of piece i pops at piece i+10.
            # v tiles sit a bit later than in v3 so the hlo DMAs (behind
            # hhi in the load order) have landed by the time the in-order
            # PE stream reaches them.
            qk_group(2, 0, V)                 # k01 cols 0:512
            qk_group(0, 0, V)                 # q01 cols 0:512
            for t in range(4):
                piece(0, t, 0, q1=512)        # needs only the two groups above
            qk_group(0, 1, V)                 # q01 cols 512:1024
            for t in range(4):
                piece(0, t, 0, q0=512)
            qk_group(2, 1, V)
            piece(0, 4, 0)
            v_tile(0)
            v_tile(1)
            piece(0, 5, 0)
            piece(0, 6, 0)
            v_tile(2)
            v_tile(3)
            piece(0, 7, 0)
            qk_group(0, 2, V)
            qk_group(0, 3, V)
            for t in range(2):
                piece(0, t, 1)
            v_tile(4)
            v_tile(5)
            for t in range(2, 4):
                piece(0, t, 1)
            v_tile(6)
            v_tile(7)
            for t in range(4, 6):
                piece(0, t, 1)
            qk_group(2, 2, V)
            qk_group(2, 3, V)
            for t in range(6, 10):
                piece(0, t, 1)
            for st in range(8, 11):
                v_tile(st)
            for t in range(10, 16):
                piece(0, t, 1)
            for st in range(11, 16):
                v_tile(st)
            for t in range(8):
                piece(1, t, 0)
            for t in range(16):
                piece(1, t, 1)
            qk_group(1, 0, V)                 # q23 cols 0:512
            qk_group(3, 0, V)                 # k23 cols 0:512
            qk_group(1, 1, V)
            qk_group(3, 1, V)
            for t in range(8):
                piece(2, t, 0)
            qk_group(1, 2, V)
            qk_group(3, 2, V)
            qk_group(1, 3, V)
            qk_group(3, 3, V)
            for t in range(16):
                piece(2, t, 1)
            for t in range(8):
                piece(3, t, 0)
            for t in range(16):
                piece(3, t, 1)
            for pc in pending:
                tail(*pc)
            pending.clear()
            # out flushes last: their waits must not block any queue that
            # still has exp/mask/transpose work (in-order sequencers).
            oflush(0, 7, nc.sync)
            oflush(7, 14, nc.scalar)
            oflush(14, 16, nc.sync)
            if DEBUG_DUMPS:
                nc.sync.dma_start(dbg_qk[:, :, :], qk_sb[:, :, :])
                nc.sync.dma_start(dbg_v[:, :, :, :], v_sb[:, :, :, :])
                nc.sync.dma_start(dbg_ao[:, :, :, :], ao_sb[:, :, :, :])
                nc.sync.dma_start(dbg_aoT[:, :, :, :], aoT_sb[:, :, :, :])
    return nc


_NC = None


def _get_nc():
    global _NC
    if _NC is None:
        _NC = build_program()
    return _NC


# ---------------------------------------------------------------- host prep

def make_in_maps(hidden_states, c_attn_w, c_attn_b, c_proj_w):
    import ml_dtypes
    bf16 = ml_dtypes.bfloat16
    f8 = mybir.dt.np(F8)

    def pack_hilo(arr):
        # [1024, N] f32 -> hi/lo fp8 DoubleRow packs [128, CP, 2, N]
        hi = arr.astype(f8)
        lo = (arr - hi.astype(np.float32)).astype(f8)
        out = []
        for part in (hi, lo):
            p = part.reshape(CP, 2, 128, -1).transpose(2, 0, 1, 3)
            out.append(np.ascontiguousarray(p))
        return out

    first_end = S // 3
    second_end = 2 * S // 3
    pos = np.arange(S)
    regions = [pos < first_end,
               (pos >= first_end) & (pos < second_end),
               pos >= second_end]
    mult = np.ones((H, S), dtype=np.float64)
    for h, r in HEAD_REGION.items():
        mult[h] = 1.0 + (FOCUS - 1.0) * regions[r].astype(np.float64)
    logm = np.log(mult).astype(np.float32)  # [H, S]

    p = np.arange(128)[:, None]
    j = np.arange(128)[None, :]
    diag = (j >= p).astype(np.float32)  # 0/1 keep-mask, applied post-exp

    in_maps = []
    for c in range(NCORES):
        b, g = divmod(c, GROUPS)
        h0 = HPC * g
        cs = slice(256 * g, 256 * g + 256)
        wq = c_attn_w[:, cs]
        wk = c_attn_w[:, 1024:2048][:, cs]
        wv = c_attn_w[:, 2048:3072][:, cs]
        # w column blocks: w01=[q01|k01], w23=[q23|k23], wv
        w01 = np.concatenate([wq[:, 0:128], wk[:, 0:128]], axis=1)
        w23 = np.concatenate([wq[:, 128:256], wk[:, 128:256]], axis=1)
        bqk = np.concatenate(
            [c_attn_b[cs], c_attn_b[1024:2048][cs]]
        ).reshape(4, 128).T.copy().astype(np.float32)
        # pw2[p, j, e]: head pair j=(2j, 2j+1); p<64 -> head 2j row p,
        # p>=64 -> head 2j+1 row p-64  (matches aoT partition layout)
        pw = c_proj_w[64 * h0:64 * h0 + 256, :].reshape(2, 128, D)
        pw = np.ascontiguousarray(pw.transpose(1, 0, 2))
        lm = logm[h0:h0 + HPC].reshape(HPC, S // 128, 128)
        lm = np.ascontiguousarray(lm.transpose(2, 0, 1)).astype(np.float32)
        h_hi, h_lo = pack_hilo(np.ascontiguousarray(hidden_states[b].T))
        im = {"bqk": bqk, "projw": pw.astype(bf16),
              "diag_mask": diag.astype(bf16), "logmult": lm}
        for name, wblk in (("w01", w01), ("w23", w23), ("wv", wv)):
            whi_, wlo_ = pack_hilo(WSCALE * wblk)
            im[f"{name}_hi"] = np.ascontiguousarray(whi_.reshape(128, -1))
            im[f"{name}_lo"] = np.ascontiguousarray(wlo_.reshape(128, -1))
        for i in range(4):
            cols = slice(512 * i, 512 * i + 512)
            im[f"h_hi{i}"] = np.ascontiguousarray(h_hi[:, :, :, cols])
            im[f"h_lo{i}"] = np.ascontiguousarray(h_lo[:, :, :, cols])
        in_maps.append(im)
    return in_maps


def run_cores(in_maps, trace=False, **kw):
    from concourse.bass_utils import run_bass_kernel_spmd
    nc = _get_nc()
    return run_bass_kernel_spmd(nc, in_maps, core_ids=list(range(NCORES)),
                                trace=trace, **kw)


def kernel(hidden_states, c_attn_w, c_attn_b, c_proj_w, c_proj_b):
    hidden_states = np.asarray(hidden_states, dtype=np.float32)
    c_attn_w = np.asarray(c_attn_w, dtype=np.float32)
    c_attn_b = np.asarray(c_attn_b, dtype=np.float32)
    c_proj_w = np.asarray(c_proj_w, dtype=np.float32)
    c_proj_b = np.asarray(c_proj_b, dtype=np.float32)

    in_maps = make_in_maps(hidden_states, c_attn_w, c_attn_b, c_proj_w)
    res = run_cores(in_maps)
    out = np.zeros((B, S, D), dtype=np.float32)
    for c in range(NCORES):
        out[c // GROUPS] += np.asarray(res.results[c]["out"], dtype=np.float32)
    # v-bias passes through the attention average exactly (rows sum to 1),
    # so it folds into a constant along with c_proj_b.
    out += (c_proj_b + c_attn_b[2048:3072] @ c_proj_w)[None, None, :]
    return out


# revision 25
# speedup vs baseline: 1.1314x; 1.1017x over previous
"""DivergentAttention Trainium2 kernel (8 NeuronCores, Bass/Tile), v5.

Problem: GPT-2 style causal self-attention (B=2, S=2048, D=1024, H=16,
hd=64) where heads 0/1/2 re-weight their attention toward a token region
(first/middle/last third of the sequence) with factor 1.6 and renormalize.

Identity: softmax(s)*m / sum(softmax(s)*m) == softmax(s + log m): the region
reweight folds into an additive per-(head, key) bias on the scores. Scores
are small (|s|<~5) so the max-subtraction pass is skipped.

Sharding: core c handles batch c//4 and heads [4*(c%4), 4*(c%4)+4); host
sums the 8 f32 c_proj partials and adds c_proj_b + c_attn_b_v @ c_proj_w
(the v-bias passes through the attention average exactly, so it folds into
a host-side constant).

v5 design notes (cost-model driven; ACT exp stream is the bottleneck):
  - QKV projection in fp8e4m3 DoubleRow (0.5 cyc/col). q/k use 2-term
    hi/lo compensation (whi+wlo)@hhi -- score noise is dominated by the
    later fp8 re-quantization of q/k anyway; v keeps 3 terms
    (whi@hhi + whi@hlo + wlo@hhi) since v enters the output directly.
    Weights host-prescaled by 64 (fp8e4m3 subnormal cutoff); q/k copies
    descale via DVE tensor_scalar(mult, add); the v-path descale cancels
    in the softmax normalization (denominator ones-column = 64).
  - Inputs are split into per-chunk DRAM tensors (w01|w23|wv hi/lo,
    hidden col-chunks) so every DMA is a whole contiguous tensor with
    >=512B descriptor runs (sub-512B runs pay a 2x DMA latency
    penalty), loaded in critical-path order: bqk, w01 hi+lo, hhi cols
    0:512 -> first score pieces (and the ACT exp stream) start ~5us in.
  - Scores in fp8 DoubleRow: q/k stripes each followed by a ZEROED
    stripe so both slot-1 operands are benign.
  - AV is FLIPPED: out[q-tile 128, 65] = attnT_tile.T @ [v | 1]: 65
    moving cols per (q,k) tile pair; the denominator (col 64) lands on
    the same partitions as q, so normalization is a per-partition
    reciprocal + tensor_scalar_mul on DVE.
  - Per-(head,q-tile) accumulators are packed 7-per-PSUM-bank at 65*4B
    stride; banks are DVE-memset and all AV matmuls use start=False.
    Banks drain only once quiescent.
  - ao[q, hd] is DMA-xbar-transposed to aoT[hd, q] per (q-tile,
    head-pair) for c_proj; c_proj PSUM results are DMAed straight to
    DRAM as f32 (no PSUM->SBUF copy), host does the final reduce. Out
    DMAs ride sync (and scalar only once the exp stream is emitted --
    a waiting DMA occupies its queue's sequencer and would stall exp
    decode). gpsimd/SWDGE never touches PSUM.
  - ACT runs ONLY the exp stream; causal 0/1 diag mask is applied
    post-exp on DVE (bf16, all-SBUF, 2x perf mode); GPSIMD only issues
    SWDGE DMAs for non-PSUM traffic.
  - Emission order IS the dependency order (Tile derives deps from
    program order) and engine-queue priority; every v_tile(t) precedes
    the first tail that reads it (build asserts this invariant).
"""

import numpy as np

import concourse.bass as bass
import concourse.tile as tile
from concourse import mybir
from concourse import bass_utils, bass2jax

# ---------------------------------------------------------------- constants
B, S, D, H, HD = 2, 2048, 1024, 16, 64
NCORES = 8
HPC = 4              # heads per core
GROUPS = 4           # head groups
FOCUS = 1.6
HEAD_REGION = {0: 0, 1: 1, 2: 2}
BF = mybir.dt.bfloat16
F32 = mybir.dt.float32
F8 = mybir.dt.float8e4
NT = S // 128         # 16
KO = D // 128         # 8
CP = 4                # DoubleRow chunk-pairs (256 logical rows each)
WSCALE = 64.0         # c_attn_w prescale so fp8e4m3 stays out of subnormals
QK_TERMS3 = False     # 3-term q/k error compensation (v is always 3-term)
DEBUG_DUMPS = False   # add intermediate-tensor DRAM dumps (debugging only)
# w column order inside w01/w23: [q01|k01] and [q23|k23]; nt (bqk order)
# 0=q01 1=q23 2=k01 3=k23 -> (tensor, col0)
WSLOT = {0: (0, 0), 2: (0, 128), 1: (1, 0), 3: (1, 128)}
# drains are bank-granular: reading a PSUM bank while matmuls still
# accumulate into other columns of it corrupts the accumulation (hw
# read-during-accumulate hazard), so a bank drains only once quiescent.
BATCHES = ((0, 7), (7, 14), (14, 16))
DRAIN_T = {6: 0, 13: 1, 15: 2}               # tail t -> batch index
BANK0 = (0, 7, 14)
LAST_IN_BANK = (True, True, True)

# ------------------------------------------------- walrus multi-wait fixup
# This container's walrus accepts only ONE sync-wait per TPB instruction,
# but Tile attaches one wait per dependency proc. Rewrite the BIR JSON just
# before walrus: hoist all-but-one wait of a multi-wait instruction onto
# standalone same-engine NoOps inserted immediately before it (same-engine
# program order is preserved, so semantics are unchanged).
try:
    import orjson as _json
except ImportError:  # pragma: no cover
    import json as _json

_orig_compile_bir_kernel = bass_utils.compile_bir_kernel
_wfix_counter = [0]


def _fix_bir(bir_json):
    d = _json.loads(bir_json)
    changed = False
    for fn in d.get("functions", []):
        for blk in fn.get("blocks", []):
            out = []
            for inst in blk.get("instructions", []):
                si = inst.get("sync_info")
                if si:
                    waits = si.get("on_wait") or []
                    if len(waits) > 1:
                        changed = True
                        for w in waits[:-1]:
                            _wfix_counter[0] += 1
                            nop = {
                                "engine": inst["engine"],
                                "ins": [],
                                "name": f"I-wfix-{_wfix_counter[0]}",
                                "opcode": "NoOp",
                                "outs": [],
                                "sync_info": {"on_update": [], "on_wait": [w]},
                            }
                            if "debug" in inst:
                                nop["debug"] = inst["debug"]
                            out.append(nop)
                        si["on_wait"] = waits[-1:]
                out.append(inst)
            blk["instructions"] = out
    return _json.dumps(d) if changed else bir_json


def _patched_compile_bir_kernel(bir_json, tmpdir, neff_name="file.neff"):
    return _orig_compile_bir_kernel(_fix_bir(bir_json), tmpdir, neff_name=neff_name)


def _install_waitfix():
    bass_utils.compile_bir_kernel = _patched_compile_bir_kernel
    bass2jax.compile_bir_kernel = _patched_compile_bir_kernel


_install_waitfix()

# ---------------------------------------------------------------- program


def build_program():
    """One SPMD Bass program; per-core differences come in via inputs."""
    nc = bass.Bass()

    # hi/lo fp8 split of hiddenT and 64*c_attn_w, DoubleRow-packed:
    # [p, cp, slot, col] holds logical contraction row 256*cp + 128*slot + p.
    # All inputs are flat [128, bytes] so each DMA is one contiguous tensor.
    w01_hi = nc.dram_tensor("w01_hi", [128, CP * 2 * 256], F8, kind="ExternalInput")
    w01_lo = nc.dram_tensor("w01_lo", [128, CP * 2 * 256], F8, kind="ExternalInput")
    w23_hi = nc.dram_tensor("w23_hi", [128, CP * 2 * 256], F8, kind="ExternalInput")
    w23_lo = nc.dram_tensor("w23_lo", [128, CP * 2 * 256], F8, kind="ExternalInput")
    wv_hi = nc.dram_tensor("wv_hi", [128, CP * 2 * 256], F8, kind="ExternalInput")
    wv_lo = nc.dram_tensor("wv_lo", [128, CP * 2 * 256], F8, kind="ExternalInput")
    h_chunk = {}
    for hl in ("hi", "lo"):
        for i in range(4):
            h_chunk[hl, i] = nc.dram_tensor(
                f"h_{hl}{i}", [128, CP, 2, 512], F8, kind="ExternalInput")
    bqk = nc.dram_tensor("bqk", [128, 4], F32, kind="ExternalInput")
    projw = nc.dram_tensor("projw", [128, 2, D], BF, kind="ExternalInput")
    diag_mask = nc.dram_tensor("diag_mask", [128, 128], BF, kind="ExternalInput")
    logmult = nc.dram_tensor("logmult", [128, HPC, NT], F32, kind="ExternalInput")
    out = nc.dram_tensor("out", [S, D], BF, kind="ExternalOutput")
    if DEBUG_DUMPS:
        dbg_qk = nc.dram_tensor("dbg_qk", [128, 8, S], F8, kind="ExternalOutput")
        dbg_v = nc.dram_tensor("dbg_v", [128, NT, HPC, 65], BF,
                               kind="ExternalOutput")
        dbg_ao = nc.dram_tensor("dbg_ao", [128, NT, 2, 128], BF,
                                kind="ExternalOutput")
        dbg_aoT = nc.dram_tensor("dbg_aoT", [128, NT, 2, 128], BF,
                                 kind="ExternalOutput")
        dbg_at = nc.dram_tensor("dbg_at", [128, 4, 512], BF,
                                kind="ExternalOutput")
        dbg_av = nc.dram_tensor("dbg_av", [128, 455], F32,
                                kind="ExternalOutput")

    with tile.TileContext(nc) as tc:
        with tc.tile_pool(name="persist", bufs=1) as persist, \
             tc.tile_pool(name="p2at", bufs=26) as p2at, \
             tc.tile_pool(name="p2rec", bufs=8) as p2rec, \
             tc.tile_pool(name="p2sc", bufs=2, space="PSUM") as p2sc, \
             tc.tile_pool(name="p2av", bufs=2, space="PSUM") as p2av, \
             tc.tile_pool(name="mix", bufs=2, space="PSUM") as mix:

            # fp8 q/k for DoubleRow scores: each stripe is followed by a
            # ZEROED stripe so both DoubleRow slot-1 operands are benign:
            # the k-side slot-1 weights are 0.0 and the q-side slot-1 data
            # is 0.0 (never a NaN bit pattern from uninitialized SBUF).
            # snt: q01=0, q23=2, k01=4, k23=6; zeros at 1, 3, 5, 7.
            qk_sb = persist.tile([128, 8, S], F8)            # 2 MB
            v_sb = persist.tile([128, NT, HPC, 65], BF)      # ~1.06 MB
            ao_sb = persist.tile([128, NT, 2, 128], BF)      # 1 MB
            aoT_sb = persist.tile([128, NT, 2, 128], BF)     # 1 MB
            bqk_sb = persist.tile([128, 4], F32)
            pw_sb = persist.tile([128, 2, D], BF)
            dm_sb = persist.tile([128, 128], BF)
            lm_sb = persist.tile([128, HPC, NT], F32)
            hhi = persist.tile([128, CP, 2, S], F8)          # 2 MB
            hlo = persist.tile([128, CP, 2, S], F8)          # 2 MB
            # w SBUF tiles: [128, pair(01/23/v), cp, slot, 256]
            w_hi = persist.tile([128, 3, CP, 2, 256], F8)
            w_lo = persist.tile([128, 3, CP, 2, 256], F8)
            o_big = persist.tile([128, NT, D], BF)           # 4 MB out stage
            warm_sb = persist.tile([128, 2, 1024], F8)       # PE warmup zeros

            # v_aug ones column = WSCALE so the w-prescale cancels in the
            # softmax normalization (num and den both carry WSCALE).
            nc.vector.memset(warm_sb[:, :, :].bitcast(F32), 0.0)
            nc.vector.memset(v_sb[:, :, :, 64:65], WSCALE)
            for z in (1, 3, 5, 7):
                nc.vector.memset(qk_sb[:, z, :].bitcast(F32), 0.0)

            def wflat(dst, pair):
                return dst[:, pair, :, :, :].rearrange("p a s c -> p (a s c)")

            # ---- loads (critical-path ordered) ----
            # HWDGE serializes issue (~630ns each) and the DMA engines
            # serialize transfers, so loads are few, whole-tensor DMAs.
            # ALL on sync: a waiting/issuing DMA occupies its queue's
            # sequencer, so the scalar (ACT) queue must stay clear for exp
            # decode and the gpsimd (Pool) queue for the diag masks.
            htile = {"hi": hhi, "lo": hlo}

            def hload(hl, i):
                nc.sync.dma_start(
                    htile[hl][:, :, :, 512 * i:512 * i + 512],
                    h_chunk[hl, i][:, :, :, :])

            nc.sync.dma_start(bqk_sb, bqk[:, :])
            nc.sync.dma_start(wflat(w_hi, 0), w01_hi[:, :])
            nc.sync.dma_start(wflat(w_lo, 0), w01_lo[:, :])
            hload("hi", 0)
            nc.gpsimd.dma_start(dm_sb, diag_mask[:, :])
            nc.gpsimd.dma_start(lm_sb, logmult[:, :, :])
            hload("hi", 1)
            nc.sync.dma_start(wflat(w_hi, 2), wv_hi[:, :])
            nc.sync.dma_start(wflat(w_lo, 2), wv_lo[:, :])
            hload("lo", 0)
            hload("hi", 2)
            hload("hi", 3)
            hload("lo", 1)
            nc.sync.dma_start(wflat(w_hi, 1), w23_hi[:, :])
            nc.sync.dma_start(wflat(w_lo, 1), w23_lo[:, :])
            hload("lo", 2)
            hload("lo", 3)
            nc.sync.dma_start(pw_sb, projw[:, :, :])

            # ---- phase-1 building blocks ----
            _gq = [0]

            def qk_group(nt, sc, eng):
                # one [128, 512] output group; the PSUM->SBUF copy descales
                # by 1/WSCALE and adds the per-partition bias.
                _gq[0] += 1
                pair, c0 = WSLOT[nt]
                ps = mix.tile([128, 512], F32, tag="m", name=f"qk{_gq[0]}")
                terms = ((w_hi, hhi), (w_lo, hhi))
                if QK_TERMS3:
                    terms = terms + ((w_hi, hlo),)
                n = len(terms) * CP
                i = 0
                for wt, ht in terms:
                    for cp in range(CP):
                        nc.tensor.matmul(
                            ps,
                            wt[:, pair, cp, :, c0:c0 + 128],
                            ht[:, cp, :, 512 * sc:512 * sc + 512],
                            start=(i == 0), stop=(i == n - 1),
                            perf_mode=mybir.MatmulPerfMode.DoubleRow,
                        )
                        i += 1
                snt = (0, 2, 4, 6)[nt]   # storage stripe (zeros odd)
                eng.tensor_scalar(
                    qk_sb[:, snt, 512 * sc:512 * sc + 512],
                    ps,
                    1.0 / WSCALE,
                    bqk_sb[:, nt:nt + 1],
                    mybir.AluOpType.mult,
                    mybir.AluOpType.add,
                )

            v_emitted = set()

            def v_tile(st):
                v_emitted.add(st)
                ps = mix.tile([128, 512], F32, tag="m", name=f"v{st}")
                terms = ((w_hi, hhi), (w_lo, hhi), (w_hi, hlo))
                n = len(terms) * CP
                i = 0
                for wt, ht in terms:
                    for cp in range(CP):
                        nc.tensor.matmul(
                            ps[:, 0:256],
                            ht[:, cp, :, 128 * st:128 * st + 128],
                            wt[:, 2, cp, :, :],
                            start=(i == 0), stop=(i == n - 1),
                            perf_mode=mybir.MatmulPerfMode.DoubleRow,
                        )
                        i += 1
                # v bias is folded into the host-side output constant
                # (attention rows sum to 1); GPSIMD cannot read PSUM -> DVE
                nc.vector.tensor_copy(
                    v_sb[:, st, :, 0:64],
                    ps[:, 0:256].rearrange("p (h d) -> p h d", d=64),
                )

            # ---- phase-2 building blocks ----
            av_tiles = {}

            def get_av(lh, tau):
                # hw PSUM accumulation groups don't interleave within a
                # bank: pre-zero the bank and accumulate with start=False.
                bank = 0 if tau < 7 else (1 if tau < 14 else 2)
                if (lh, bank) not in av_tiles:
                    pool = p2av if bank < 2 else mix
                    tag = "av" if bank < 2 else "m"
                    tile_ = pool.tile(
                        [128, 512], F32, tag=tag, name=f"av{lh}{bank}")
                    nc.vector.memset(tile_[:, :], 0.0)
                    av_tiles[(lh, bank)] = tile_
                return av_tiles[(lh, bank)], 7 * (bank > 0) + 7 * (bank > 1)

            def cproj(tau):
                for ec in range(2):
                    # PSUM slots freed upstream become c_proj parallelism:
                    # tau<7 runs while h3 still owns avB + avC (mix slot 1 +
                    # the just-freed p2av slot A); tau>=7 additionally uses
                    # the idle score pool (exp stream is ending).
                    wide = False
                    if tau < 7:
                        pool, tg = (mix, "m") if ec == 0 else (p2av, "av")
                    elif (2 * tau + ec) % 4 == 0:
                        pool, tg = mix, "m"
                    elif (2 * tau + ec) % 4 == 2:
                        pool, tg, wide = p2sc, "sc", True
                    else:
                        pool, tg = p2av, "av"
                    if wide:
                        ps = pool.tile([128, 1024], F32, tag=tg,
                                       name=f"pr{tau}{ec}")[:, 0:512]
                    else:
                        ps = pool.tile([128, 512], F32, tag=tg,
                                       name=f"pr{tau}{ec}")
                    for j in range(2):
                        nc.tensor.matmul(
                            ps,
                            aoT_sb[:, tau, j, :],
                            pw_sb[:, j, 512 * ec:512 * ec + 512],
                            start=(j == 0), stop=(j == 1),
                        )
                    # DMA cannot read PSUM: bounce through the o_big stage.
                    # tau>=7 drains fire after every exp is emitted, so ACT
                    # (idle by then) absorbs half the copies; tau<7 drains
                    # still race the last lh3 exps -> DVE only.
                    dst = o_big[:, tau, 512 * ec:512 * ec + 512]
                    if tau >= 7 and ec == 1:
                        nc.scalar.copy(dst, ps)
                    else:
                        nc.vector.tensor_copy(dst, ps)
                # one [128, 1024] out DMA per q-tile, pipelined with the
                # copies of the next tile. scalar queue only once the exp
                # stream is fully emitted (tau>=7 drains).
                oq = nc.scalar if tau >= 7 and tau % 2 else nc.sync
                oq.dma_start(
                    out[128 * tau:128 * tau + 128, :], o_big[:, tau, :])

            def drain(lh, b):
                # all q-tiles of this batch fully accumulated: reciprocal
                # of the denominator column, per-partition scale into ao_sb.
                t0, t1 = BATCHES[b]
                nb = t1 - t0
                bank = 0 if t0 < 7 else (1 if t0 < 14 else 2)
                if LAST_IN_BANK[b]:
                    av = av_tiles.pop((lh, bank))
                else:
                    av = av_tiles[(lh, bank)]
                av = av[:, 65 * (t0 - BANK0[b]):]
                j, hp = lh // 2, lh % 2
                if DEBUG_DUMPS and lh == 0 and b == 0:
                    avcp = persist.tile([128, 455], F32)
                    nc.vector.tensor_copy(avcp, av[:, 0:455])
                    nc.sync.dma_start(dbg_av[:, :], avcp[:, :])
                rec = p2rec.tile([128, 8], F32, tag="rec")
                den = av[:, 0:65 * nb].rearrange(
                    "p (n c) -> p n c", c=65)[:, :, 64:65]
                nc.vector.reciprocal(rec[:, 0:nb], den)
                for k in range(nb):
                    tau = t0 + k
                    # lh3 late drains run after the exp stream has ended,
                    # so ACT can absorb half those normalize multiplies;
                    # bank-0 drains still race the last exps -> DVE.
                    if lh == 3 and b >= 1 and k % 2 == 1:
                        nc.scalar.mul(
                            ao_sb[:, tau, j, 64 * hp:64 * hp + 64],
                            av[:, 65 * k:65 * k + 64],
                            rec[:, k:k + 1],
                        )
                    else:
                        nc.vector.tensor_scalar_mul(
                            ao_sb[:, tau, j, 64 * hp:64 * hp + 64],
                            av[:, 65 * k:65 * k + 64],
                            rec[:, k:k + 1],
                        )
                if hp == 1:
                    # both heads of pair j drained: transpose ao[q, hd] ->
                    # aoT[hd, q] on the DMA xbar; after the last pair,
                    # this q-tile's c_proj is fully unblocked.
                    for k in range(nb):
                        tau = t0 + k
                        nc.sync.dma_start_transpose(
                            aoT_sb[:, tau, j, :], ao_sb[:, tau, j, :])
                    if lh == 3:
                        for k in range(nb):
                            cproj(t0 + k)

            def tail(lh, t, q0, width, at_sb):
                # exp consumers: causal 0/1 mask on the diagonal block
                # (GPSIMD, all-SBUF) + flipped AV accumulation.
                if q0 == 128 * t:
                    nc.gpsimd.tensor_mul(
                        out=at_sb[:, 0:128], in0=at_sb[:, 0:128],
                        in1=dm_sb,
                    )
                if DEBUG_DUMPS and lh == 0 and t < 4 and q0 < 512:
                    nc.sync.dma_start(dbg_at[:, t, :], at_sb[:, 0:512])
                assert t in v_emitted, (
                    f"tail({lh},{t}) before v_tile({t}): program-order "
                    "dependency violation (reads uninitialized v_sb)")
                v_aug = v_sb[:, t, lh, :]
                for tau in range(q0 // 128, (q0 + width) // 128):
                    av, base = get_av(lh, tau)
                    col = 65 * (tau - base)
                    off = 128 * tau - q0
                    nc.tensor.matmul(
                        av[:, col:col + 65],
                        at_sb[:, off:off + 128],
                        v_aug,
                        start=False, stop=(t == tau),
                        skip_group_check=True,
                    )
                full = q0 + width == (1024 if t < 8 else 2048)
                if full and t in DRAIN_T and (t > 6 or q0 < 1024):
                    drain(lh, DRAIN_T[t])

            pending = []

            def piece(lh, t, hf, q0=None, q1=None):
                if q0 is None:
                    q0 = max(128 * t, 1024 * hf)
                if q1 is None:
                    q1 = 1024 * (hf + 1)
                if q0 >= q1:
                    return
                width = q1 - q0
                bp = 64 * (lh % 2)
                q_nt = 2 * (lh // 2)          # slots (q stripe, zeros)
                k_nt = 4 + 2 * (lh // 2)      # slots (k stripe, zeros)
                lhsT_k = qk_sb[bp:bp + 64, k_nt:k_nt + 2,
                               128 * t:128 * t + 128]
                sc_ps = p2sc.tile([128, 1024], F32, tag="sc")
                off = 0
                while off < width:
                    w512 = min(512, width - off)
                    nc.tensor.matmul(
                        sc_ps[:, off:off + w512],
                        lhsT_k,
                        qk_sb[bp:bp + 64, q_nt:q_nt + 2,
                              q0 + off:q0 + off + w512],
                        start=True, stop=True,
                        perf_mode=mybir.MatmulPerfMode.DoubleRow,
                    )
                    off += w512
                at_sb = p2at.tile([128, 1024], BF, tag="attnT")
                nc.scalar.activation(
                    at_sb[:, :width], sc_ps[:, :width],
                    mybir.ActivationFunctionType.Exp,
                    bias=lm_sb[:, lh, t:t + 1], scale=0.125,
                )
                pending.append((lh, t, q0, width, at_sb))
                if len(pending) > 9:
                    tail(*pending.pop(0))

            # ---- interleaved emission: program order is engine priority ----
            # PE p-state warmup: the cost model runs PE at 0.65/1.2 GHz
            # until it has been busy ~3us, and the ramp clock resets on
            # idle. Zero-input dummy matmuls keep PE busy from ~1.3us so
            # the first real matmuls (~7us, DMA-bound) run at 2.4 GHz.
            warm_ps = p2sc.tile([128, 1024], F32, tag="sc", name="warm")
            for _ in range(12):
                nc.tensor.matmul(
                    warm_ps[:, 0:512],
                    warm_sb[0:64, :, 0:128],
                    warm_sb[0:64, :, 0:512],
                    start=True, stop=True,
                    perf_mode=mybir.MatmulPerfMode.DoubleRow,
                )
            V = nc.vector
            # NOTE: tails (av matmuls) consume v_sb, and Tile derives
            # dependencies from program order -- every v_tile(st) must be
            # emitted BEFORE the first tail that reads v_sb[:, st].
            # With pending depth 10, tail of piece i pops at piece i+10.
            # v tiles sit a bit later than in v3 so the hlo DMAs (behind
            # hhi in the load order) have landed by the time the in-order
            # PE stream reaches them.
            qk_group(2, 0, V)                 # k01 cols 0:512
            qk_group(0, 0, V)                 # q01 cols 0:512
            for t in range(4):
                piece(0, t, 0, q1=512)        # needs only the two groups above
            qk_group(0, 1, V)                 # q01 cols 512:1024
            for t in range(4):
                piece(0, t, 0, q0=512)
            qk_group(2, 1, V)
            piece(0, 4, 0)
            v_tile(0)
            v_tile(1)
            piece(0, 5, 0)
            piece(0, 6, 0)
            v_tile(2)
            v_tile(3)
            piece(0, 7, 0)
            qk_group(0, 2, V)
            qk_group(0, 3, V)
            for t in range(2):
                piece(0, t, 1)
            v_tile(4)
            v_tile(5)
            for t in range(2, 4):
                piece(0, t, 1)
            v_tile(6)
            v_tile(7)
            for t in range(4, 6):
                piece(0, t, 1)
            qk_group(2, 2, V)
            qk_group(2, 3, V)
            for t in range(6, 10):
                piece(0, t, 1)
            for st in range(8, 11):
                v_tile(st)
            for t in range(10, 16):
                piece(0, t, 1)
            for st in range(11, 16):
                v_tile(st)
            for t in range(8):
                piece(1, t, 0)
            for t in range(16):
                piece(1, t, 1)
            qk_group(1, 0, V)                 # q23 cols 0:512
            qk_group(3, 0, V)                 # k23 cols 0:512
            qk_group(1, 1, V)
            qk_group(3, 1, V)
            for t in range(8):
                piece(2, t, 0)
            qk_group(1, 2, V)
            qk_group(3, 2, V)
            qk_group(1, 3, V)
            qk_group(3, 3, V)
            for t in range(16):
                piece(2, t, 1)
            for t in range(8):
                piece(3, t, 0)
            for t in range(16):
                piece(3, t, 1)
            for pc in pending:
                tail(*pc)
            pending.clear()
            if DEBUG_DUMPS:
                nc.sync.dma_start(dbg_qk[:, :, :], qk_sb[:, :, :])
                nc.sync.dma_start(dbg_v[:, :, :, :], v_sb[:, :, :, :])
                nc.sync.dma_start(dbg_ao[:, :, :, :], ao_sb[:, :, :, :])
                nc.sync.dma_start(dbg_aoT[:, :, :, :], aoT_sb[:, :, :, :])
    return nc


_NC = None


def _get_nc():
    global _NC
    if _NC is None:
        _NC = build_program()
    return _NC


# ---------------------------------------------------------------- host prep

def make_in_maps(hidden_states, c_attn_w, c_attn_b, c_proj_w):
    import ml_dtypes
    bf16 = ml_dtypes.bfloat16
    f8 = mybir.dt.np(F8)

    def pack_hilo(arr):
        # [1024, N] f32 -> hi/lo fp8 DoubleRow packs [128, CP, 2, N]
        hi = arr.astype(f8)
        lo = (arr - hi.astype(np.float32)).astype(f8)
        out = []
        for part in (hi, lo):
            p = part.reshape(CP, 2, 128, -1).transpose(2, 0, 1, 3)
            out.append(np.ascontiguousarray(p))
        return out

    first_end = S // 3
    second_end = 2 * S // 3
    pos = np.arange(S)
    regions = [pos < first_end,
               (pos >= first_end) & (pos < second_end),
               pos >= second_end]
    mult = np.ones((H, S), dtype=np.float64)
    for h, r in HEAD_REGION.items():
        mult[h] = 1.0 + (FOCUS - 1.0) * regions[r].astype(np.float64)
    logm = np.log(mult).astype(np.float32)  # [H, S]

    p = np.arange(128)[:, None]
    j = np.arange(128)[None, :]
    diag = (j >= p).astype(np.float32)  # 0/1 keep-mask, applied post-exp

    in_maps = []
    for c in range(NCORES):
        b, g = divmod(c, GROUPS)
        h0 = HPC * g
        cs = slice(256 * g, 256 * g + 256)
        wq = c_attn_w[:, cs]
        wk = c_attn_w[:, 1024:2048][:, cs]
        wv = c_attn_w[:, 2048:3072][:, cs]
        # w column blocks: w01=[q01|k01], w23=[q23|k23], wv
        w01 = np.concatenate([wq[:, 0:128], wk[:, 0:128]], axis=1)
        w23 = np.concatenate([wq[:, 128:256], wk[:, 128:256]], axis=1)
        bqk = np.concatenate(
            [c_attn_b[cs], c_attn_b[1024:2048][cs]]
        ).reshape(4, 128).T.copy().astype(np.float32)
        # pw2[p, j, e]: head pair j=(2j, 2j+1); p<64 -> head 2j row p,
        # p>=64 -> head 2j+1 row p-64  (matches aoT partition layout)
        pw = c_proj_w[64 * h0:64 * h0 + 256, :].reshape(2, 128, D)
        pw = np.ascontiguousarray(pw.transpose(1, 0, 2))
        lm = logm[h0:h0 + HPC].reshape(HPC, S // 128, 128)
        lm = np.ascontiguousarray(lm.transpose(2, 0, 1)).astype(np.float32)
        h_hi, h_lo = pack_hilo(np.ascontiguousarray(hidden_states[b].T))
        im = {"bqk": bqk, "projw": pw.astype(bf16),
              "diag_mask": diag.astype(bf16), "logmult": lm}
        for name, wblk in (("w01", w01), ("w23", w23), ("wv", wv)):
            whi_, wlo_ = pack_hilo(WSCALE * wblk)
            im[f"{name}_hi"] = np.ascontiguousarray(whi_.reshape(128, -1))
            im[f"{name}_lo"] = np.ascontiguousarray(wlo_.reshape(128, -1))
        for i in range(4):
            cols = slice(512 * i, 512 * i + 512)
            im[f"h_hi{i}"] = np.ascontiguousarray(h_hi[:, :, :, cols])
            im[f"h_lo{i}"] = np.ascontiguousarray(h_lo[:, :, :, cols])
        in_maps.append(im)
    return in_maps


def run_cores(in_maps, trace=False, **kw):
    from concourse.bass_utils import run_bass_kernel_spmd
    nc = _get_nc()
    return run_bass_kernel_spmd(nc, in_maps, core_ids=list(range(NCORES)),
                                trace=trace, **kw)


def kernel(hidden_states, c_attn_w, c_attn_b, c_proj_w, c_proj_b):
    hidden_states = np.asarray(hidden_states, dtype=np.float32)
    c_attn_w = np.asarray(c_attn_w, dtype=np.float32)
    c_attn_b = np.asarray(c_attn_b, dtype=np.float32)
    c_proj_w = np.asarray(c_proj_w, dtype=np.float32)
    c_proj_b = np.asarray(c_proj_b, dtype=np.float32)

    in_maps = make_in_maps(hidden_states, c_attn_w, c_attn_b, c_proj_w)
    res = run_cores(in_maps)
    out = np.zeros((B, S, D), dtype=np.float32)
    for c in range(NCORES):
        out[c // GROUPS] += np.asarray(res.results[c]["out"], dtype=np.float32)
    # v-bias passes through the attention average exactly (rows sum to 1),
    # so it folds into a constant along with c_proj_b.
    out += (c_proj_b + c_attn_b[2048:3072] @ c_proj_w)[None, None, :]
    return out
